# revision 45
# baseline (speedup 1.0000x reference)
"""GCN layers (3x GCNConv + PReLU + residual + BatchNorm) on 8 TRN2 NeuronCores.

Full-input contract: kernel(**inputs) takes unsharded numpy arrays and returns
the full [50000, 64] float32 output.

Key restructuring vs the naive scheme: GCN aggregation is linear, so
  agg = segsum(w_e * (h W)[src]) = segsum(w_e * h[src]) @ W
and BatchNorm is a per-feature affine h = gs*p + gb, so
  segsum(w_e * h[src]) = gs * segsum(w_e * p[src]) + gb * deg_w[dst].
Therefore the gather table per layer is the RAW pre-BN activation p:
 - no matmul before the table AllGather,
 - the BN-stats AllReduce overlaps the entire next aggregation phase,
 - layer 0's table (= x) is precomputed on host and replicated (no
   collective at all in layer 0).

Sharding: nodes in 8 contiguous ranges (dst-sharded edges). Per layer:
 1. AllGather node-major bf16 table p [50176, 128] (64 valid cols) split in
    two pieces A/B (keeps gather idx in int16; overlaps AG-B with gathers-A)
 2. stream edges: dma_gather 256B rows table[src] -> SBUF; scatter matrices
    S (one nonzero w_e per edge row) are built ON DEVICE from a persistent
    [col, w] bf16 table via iota==col compare (2 DVE ops per 6144-edge
    chunk); per-128-edge subchunk matmuls agg^T[blk] += msg^T @ S (PSUM)
 3. transform + epilogue feature-major: hagg = gs*ragg + gb*degw (bf16),
    agg^T = W^T @ hagg^T, +bias, PReLU, +residual, with BN stats
    accumulated for free via accum_out; tiny stats AllReduce fully
    overlapped with the next layer's table AllGather + gathers.
"""

import os
import numpy as np

N_NODES = 50000
D = 64
L = 3
BN_EPS = 1e-5
N_CORES = 8
GCHUNK = 3072           # edge slots per gather chunk (24 subchunks of 128)
BLKN = 64               # dst nodes per aggregation block (S columns)
IDX_LIMIT = 32768       # int16 gather index range
N_QUEUES = 4            # SWDGE queues used round-robin for gathers
GBUFS = 8               # msg/S buffers in flight (2 per queue)

LAST_RUN = {}


def _balance_nodes(n, nsh, npad, asplit, blkn, edge_src, edge_dst):
    """Assign nodes to (core, slot) so per-(core,block) edge counts for the
    A-stream (src slot < asplit) and B-stream pack near 128-edge subchunk
    boundaries. Returns slot_of_node [n] -> core * npad + slot.

    A-half membership is fixed up front (node id < n_a nodes with the largest
    degrees would perturb streams; keep natural order so streams are
    determined before placement): nodes [0, n_a) live in A slots, rest in B.
    """
    n_cores = N_CORES
    n_a = n_cores * asplit
    a_half = np.zeros(n, bool)
    a_half[:n_a] = True

    # per-node in-edge counts split by src half
    src_in_a = a_half[edge_src]
    a_cnt = np.bincount(edge_dst[src_in_a], minlength=n).astype(np.int64)
    b_cnt = np.bincount(edge_dst[~src_in_a], minlength=n).astype(np.int64)

    nblk = npad // blkn
    nblk_a = asplit // blkn
    slots_per_bin = blkn
    # bins: core * nblk + blk.  A-nodes only into blk < nblk_a, B-nodes rest.
    nbins = n_cores * nblk
    bin_a = np.zeros(nbins, np.int64)
    bin_b = np.zeros(nbins, np.int64)
    bin_free = np.full(nbins, slots_per_bin, np.int64)
    # reserve pad slots in the globally-last block of each core
    for r in range(n_cores):
        bin_free[r * nblk + nblk - 1] -= npad - nsh
    is_a_bin = np.tile(np.arange(nblk) < nblk_a, n_cores)

    placements = np.empty(n, np.int64)  # bin id per node
    filled = {True: [], False: []}
    for half, nodes_mask in ((True, a_half), (False, ~a_half)):
        nodes = np.nonzero(nodes_mask)[0]
        order = np.argsort(-(a_cnt[nodes] + b_cnt[nodes]), kind="stable")
        nodes = nodes[order]
        sel = np.nonzero(is_a_bin == half)[0]
        ba = bin_a[sel].astype(np.float64)
        bb = bin_b[sel].astype(np.float64)
        bf = bin_free[sel].copy()
        tot_a = a_cnt[nodes].sum()
        tot_b = b_cnt[nodes].sum()
        nb = len(sel)
        # per-block-row caps aligned to 128-edge subchunk boundaries (~2%
        # slack); all 8 cores of a row share a cap so the schedule's
        # max-over-cores ceil() lands exactly on the row target
        rows = sel % nblk
        urows = np.unique(rows)
        nrows = len(urows)

        def caps(tot):
            target = int(tot * 1.02) + 8 * 128
            lo = max(1, tot // (nrows * 8 * 128))  # subchunks/bin, low tier
            n_hi = max(0, min(nrows,
                              -(-(target - nrows * 8 * lo * 128)
                                // (8 * 128))))
            row_cap = {r: (lo + (k < n_hi)) * 128
                       for k, r in enumerate(urows)}
            return np.array([row_cap[r] for r in rows], np.float64)
        cap_a = caps(tot_a)
        cap_b = caps(tot_b)
        ta = max(1.0, tot_a / nb)
        tb = max(1.0, tot_b / nb)
        for nd in nodes:
            ai, bi = a_cnt[nd], b_cnt[nd]
            over = (np.maximum(0.0, ba + ai - cap_a)
                    + np.maximum(0.0, bb + bi - cap_b))
            # 2D least-loaded with per-row cap guard
            cost = (over * 1e6
                    + (ba + ai) / ta + (bb + bi) / tb)
            cost = np.where(bf > 0, cost, np.inf)
            j = int(np.argmin(cost))
            placements[nd] = sel[j]
            ba[j] += ai
            bb[j] += bi
            bf[j] -= 1
        bin_a[sel] = ba.astype(np.int64)
        bin_b[sel] = bb.astype(np.int64)
        bin_free[sel] = bf

    # slots within a bin: in placement order
    slot_of_node = np.empty(n, np.int64)
    order = np.argsort(placements, kind="stable")
    sorted_bins = placements[order]
    starts = np.searchsorted(sorted_bins, np.arange(nbins))
    ends = np.searchsorted(sorted_bins, np.arange(nbins), side="right")
    for b in range(nbins):
        nd = order[starts[b]:ends[b]]
        core, blk = divmod(b, nblk)
        base = core * npad + blk * blkn
        slot_of_node[nd] = base + np.arange(len(nd))
    return slot_of_node


# ----------------------------------------------------------------------------
# Host-side preprocessing
# ----------------------------------------------------------------------------

def _wrap16(flat, slots):
    """Edge-slot array -> [128, slots/16] int16 'wrapped' index layout."""
    a = flat.reshape(slots // 16, 16).T.astype(np.int16)
    return np.tile(a, (8, 1))


def _preprocess(x, edge_src, edge_dst, edge_weight, W, b, prelu_a,
                bn_gamma, bn_beta, n_cores, nsh, gchunk, blkn):
    import ml_dtypes
    bf16 = ml_dtypes.bfloat16

    n = x.shape[0]
    d = x.shape[1]
    nt = (nsh + 127) // 128
    npad = nt * 128
    subc = gchunk // 128
    nblk = npad // blkn
    asplit = (IDX_LIMIT // n_cores) // 128 * 128     # 4096 local rows -> A
    bsplit = npad - asplit                            # 2176 local rows -> B
    rows_a = n_cores * asplit
    rows_b = n_cores * bsplit

    src = np.asarray(edge_src).astype(np.int64)
    dst = np.asarray(edge_dst).astype(np.int64)
    w = np.asarray(edge_weight).astype(np.float32)
    x = np.asarray(x).astype(np.float32)

    slot_of_node = _balance_nodes(n, nsh, npad, asplit, blkn, src, dst)

    s_slot = slot_of_node[src]
    s_rank = s_slot // npad
    s_loc = s_slot % npad
    inA = s_loc < asplit
    idxA = s_rank * asplit + s_loc
    idxB = s_rank * bsplit + (s_loc - asplit)
    d_slot = slot_of_node[dst]
    shard = d_slot // npad
    dst_local = d_slot % npad

    streams = []
    for sel, tix, trows in ((inA, idxA, rows_a), (~inA, idxB, rows_b)):
        per_core_edges = []
        cnts = []
        for r in range(n_cores):
            m = (shard == r) & sel
            per_core_edges.append((tix[m], dst_local[m], w[m]))
            cnts.append(np.bincount(dst_local[m] // blkn, minlength=nblk))
        nsub = np.zeros(nblk, np.int64)
        for c in cnts:
            nsub = np.maximum(nsub, (c + 127) // 128)
        sub_off = np.concatenate([[0], np.cumsum(nsub)])
        total_subs = int(sub_off[-1])
        nch = max(1, (total_subs + subc - 1) // subc)
        padded_subs = nch * subc
        slots = padded_subs * 128

        # per chunk: list of (j, blk, st, sp); done_blocks after each chunk
        sched = [[] for _ in range(nch)]
        for blk in range(nblk):
            for j in range(int(nsub[blk])):
                gsub = int(sub_off[blk]) + j
                sched[gsub // subc].append(
                    (gsub % subc, blk, j == 0, j == int(nsub[blk]) - 1))
        done = []
        for c in range(nch):
            hi = (c + 1) * subc
            done.append(int(np.searchsorted(sub_off[1:], hi, side="right")))
        drain_chunk = [int(sub_off[blk + 1] - 1) // subc if nsub[blk] else -1
                       for blk in range(nblk)]

        per_core = []
        for r in range(n_cores):
            ti, dl, wr = per_core_edges[r]
            blk = dl // blkn
            col = dl % blkn
            order = np.argsort(blk, kind="stable")
            ti, wr, blk, col = (a[order] for a in (ti, wr, blk, col))
            cnt = cnts[r]
            starts = np.concatenate([[0], np.cumsum(cnt)])
            pos = np.arange(len(ti)) - starts[blk]
            gsub = sub_off[blk] + pos // 128
            row = pos % 128
            slot = gsub * 128 + row
            rng_pad = np.random.default_rng(12345 + r)
            idx = rng_pad.integers(0, trows, slots)
            idx[total_subs * 128:] = -1   # uniform tail dummies: no descriptors
            idx[slot] = ti
            S = np.zeros((padded_subs, 128, blkn), np.float32)
            S[gsub, row, col] = wr
            St = np.ascontiguousarray(S.transpose(1, 0, 2)
                                      .reshape(128, padded_subs * blkn))
            per_core.append((_wrap16(idx, slots), St.astype(bf16)))
        streams.append(dict(nch=nch, slots=slots, sched=sched, done=done,
                            padded_subs=padded_subs, nsub=nsub,
                            drain_chunk=drain_chunk,
                            total_subs=total_subs, per_core=per_core))

    # ---- merged event schedule: advance the stream that lags in completed
    # blocks, drop epilogue chunks (and the table AllGathers) in as soon as
    # their blocks are fully drained on both streams -----------------------
    events = []
    nec = (npad + 511) // 512
    blk_per_epi = 512 // blkn
    nchA, nchB = streams[0]["nch"], streams[1]["nch"]
    ca = cb = 0
    epi_next = 0
    ag_a_emitted = False
    merged_pos = {}
    agA_after_epi = asplit // 512 - 1        # epi chunk completing col asplit
    while ca < nchA or cb < nchB:
        doneA = streams[0]["done"][ca - 1] if ca else 0
        doneB = streams[1]["done"][cb - 1] if cb else 0
        if cb >= nchB or (ca < nchA and doneA <= doneB):
            merged_pos[(0, ca)] = len(events)
            events.append(("chunk", 0, ca))
            ca += 1
        else:
            merged_pos[(1, cb)] = len(events)
            events.append(("chunk", 1, cb))
            cb += 1
        doneA = streams[0]["done"][ca - 1] if ca else 0
        doneB = streams[1]["done"][cb - 1] if cb else 0
        completed = min(doneA, doneB)
        while (epi_next < nec
               and completed * blkn >= min(npad, (epi_next + 1) * 512)):
            events.append(("epi", epi_next))
            if epi_next == agA_after_epi:
                ag_a_emitted = True
                events.append(("agA",))
            epi_next += 1
    while epi_next < nec:
        events.append(("epi", epi_next))
        if epi_next == agA_after_epi:
            ag_a_emitted = True
            events.append(("agA",))
        epi_next += 1
    assert ag_a_emitted
    events.append(("agB",))

    # first drain (in merged issue order) per block -> copy, second -> add
    copy_flags = {}   # (stream, blk) -> True if that stream's drain is a copy
    msblocks = []
    for blk in range(nblk):
        pos = []
        for s in range(2):
            dc = streams[s]["drain_chunk"][blk]
            if dc >= 0:
                pos.append((merged_pos[(s, dc)], s))
        if not pos:
            msblocks.append(blk)
            continue
        pos.sort()
        copy_flags[(pos[0][1], blk)] = True
        for _, s in pos[1:]:
            copy_flags[(s, blk)] = False

    # layer-0 gather tables (= x), replicated on every core
    tbl0A = np.zeros((rows_a, 128), np.float32)
    tbl0B = np.zeros((rows_b, 128), np.float32)
    nodes = np.arange(n)
    n_core = slot_of_node // npad
    n_loc = slot_of_node % npad
    mA = n_loc < asplit
    tbl0A[n_core[mA] * asplit + n_loc[mA], :d] = x[nodes[mA]]
    tbl0B[n_core[~mA] * bsplit + (n_loc[~mA] - asplit), :d] = x[nodes[~mA]]
    tbl0A = tbl0A.astype(bf16)
    tbl0B = tbl0B.astype(bf16)

    bT = np.ascontiguousarray(np.asarray(b, np.float32).T)
    gammaT = np.ascontiguousarray(np.asarray(bn_gamma, np.float32).T)
    betaT = np.ascontiguousarray(np.asarray(bn_beta, np.float32).T)
    prelu_rep = np.tile(np.asarray(prelu_a, np.float32).reshape(1, L),
                        (128, 1))
    Wbf = np.ascontiguousarray(np.asarray(W, np.float32)).astype(bf16)

    in_maps = []
    for r in range(n_cores):
        m = (shard == r)
        degw_pad = np.bincount(dst_local[m], weights=w[m],
                               minlength=npad).astype(np.float32)
        degw_rep = np.tile(degw_pad.reshape(1, npad), (d, 1)).astype(bf16)
        in_maps.append({
            "tbl0A": tbl0A,
            "tbl0B": tbl0B,
            "Wbf": Wbf,
            "bT": bT,
            "gammaT": gammaT,
            "betaT": betaT,
            "prelu_rep": prelu_rep,
            "degw": degw_rep,
            "srcA": streams[0]["per_core"][r][0],
            "Sa": streams[0]["per_core"][r][1],
            "srcB": streams[1]["per_core"][r][0],
            "Sb": streams[1]["per_core"][r][1],
        })

    cfg = dict(n_cores=n_cores, nsh=nsh, d=d, nt=nt, npad=npad,
               gchunk=gchunk, subc=subc, blkn=blkn, nblk=nblk,
               asplit=asplit, bsplit=bsplit, rows_a=rows_a, rows_b=rows_b,
               n_nodes=n, events=events, copy_flags=copy_flags,
               msblocks=msblocks, slot_of_node=slot_of_node,
               nchA=streams[0]["nch"], slotsA=streams[0]["slots"],
               schedA=streams[0]["sched"], subsA=streams[0]["total_subs"],
               psubsA=streams[0]["padded_subs"],
               nchB=streams[1]["nch"], slotsB=streams[1]["slots"],
               schedB=streams[1]["sched"], subsB=streams[1]["total_subs"],
               psubsB=streams[1]["padded_subs"])
    return in_maps, cfg


# ----------------------------------------------------------------------------
# Device program
# ----------------------------------------------------------------------------

def _build_nc(cfg):
    import concourse.bacc as bacc
    import concourse.tile as tile
    import concourse.mybir as mybir
    from concourse import library_config
    from concourse.masks import make_identity

    fp32 = mybir.dt.float32
    bf16 = mybir.dt.bfloat16
    i16 = mybir.dt.int16
    i32 = mybir.dt.int32
    Alu = mybir.AluOpType
    Ax = mybir.AxisListType

    n_cores = cfg["n_cores"]
    nsh, d, nt, npad = cfg["nsh"], cfg["d"], cfg["nt"], cfg["npad"]
    gchunk, subc = cfg["gchunk"], cfg["subc"]
    blkn, nblk = cfg["blkn"], cfg["nblk"]
    asplit, bsplit = cfg["asplit"], cfg["bsplit"]
    rows_a, rows_b = cfg["rows_a"], cfg["rows_b"]
    n_nodes = cfg["n_nodes"]
    slotsA, slotsB = cfg["slotsA"], cfg["slotsB"]
    nchA, nchB = cfg["nchA"], cfg["nchB"]
    psubsA, psubsB = cfg["psubsA"], cfg["psubsB"]
    i16s = gchunk // 16
    nec = (npad + 511) // 512   # 512-col epilogue chunks

    schedA = cfg["schedA"]
    schedB = cfg["schedB"]
    variant = os.environ.get("GCN_VARIANT", "")
    skip_gather = variant == "nogather"
    skip_smm = variant == "onlygather"

    nc = bacc.Bacc(None, target_bir_lowering=False, debug=False,
                   num_swdge_queues=N_QUEUES)

    tbl0A = nc.declare_dram_parameter("tbl0A", [rows_a, 128], bf16, isOutput=False)
    tbl0B = nc.declare_dram_parameter("tbl0B", [rows_b, 128], bf16, isOutput=False)
    Wbf_in = nc.declare_dram_parameter("Wbf", [L, d, d], bf16, isOutput=False)
    bT_in = nc.declare_dram_parameter("bT", [d, L], fp32, isOutput=False)
    gammaT_in = nc.declare_dram_parameter("gammaT", [d, L], fp32, isOutput=False)
    betaT_in = nc.declare_dram_parameter("betaT", [d, L], fp32, isOutput=False)
    prelu_in = nc.declare_dram_parameter("prelu_rep", [128, L], fp32, isOutput=False)
    degw_in = nc.declare_dram_parameter("degw", [d, npad], bf16, isOutput=False)
    srcA = nc.declare_dram_parameter("srcA", [128, slotsA // 16], i16, isOutput=False)
    Sa_in = nc.declare_dram_parameter("Sa", [128, psubsA * blkn], bf16, isOutput=False)
    srcB = nc.declare_dram_parameter("srcB", [128, slotsB // 16], i16, isOutput=False)
    Sb_in = nc.declare_dram_parameter("Sb", [128, psubsB * blkn], bf16, isOutput=False)
    out_ext = nc.declare_dram_parameter("out", [npad, d], fp32, isOutput=True)

    with tile.TileContext(nc) as tc:
        with (
            tc.tile_pool(name="const", bufs=1) as cpool,
            tc.tile_pool(name="state", bufs=1) as spool,
            tc.tile_pool(name="meta", bufs=1) as epool,
            tc.tile_pool(name="work", bufs=2) as wpool,
            tc.tile_pool(name="rows", bufs=1) as rpool,
            tc.tile_pool(name="msg", bufs=GBUFS) as mpool,
            tc.tile_pool(name="smat", bufs=GBUFS) as stpool,
            tc.tile_pool(name="ps", bufs=2, space="PSUM") as ppool,
            tc.tile_pool(name="psw", bufs=2, space="PSUM") as wppool,
            tc.tile_pool(name="psagg", bufs=4, space="PSUM") as apool,
            tc.tile_pool(name="dram", bufs=1, space="DRAM") as dpool,
        ):
            ragg = spool.tile([d, npad], fp32, tag="ragg")
            p_sb = spool.tile([d, npad], fp32, tag="p")
            staging = spool.tile([128, nt, 128], bf16, tag="stg")
            sumacc = spool.tile([d, 16], fp32, tag="sumacc")
            sqacc = spool.tile([d, 16], fp32, tag="sqacc")
            stat_sb = spool.tile([d, 2], fp32, tag="stat")
            stat2_sb = spool.tile([d, 2], fp32, tag="stat2")

            ident = cpool.tile([d, d], fp32, tag="ident")
            W_sb = cpool.tile([d, L * d], bf16, tag="Wsb")
            bT_sb = cpool.tile([d, L], fp32, tag="bT")
            gaT_sb = cpool.tile([d, L], fp32, tag="gaT")
            beT_sb = cpool.tile([d, L], fp32, tag="beT")
            prelu_sb = cpool.tile([128, L], fp32, tag="prelu")
            degw_sb = cpool.tile([d, npad], bf16, tag="degw")

            iA_sb = epool.tile([128, slotsA // 16], i16, tag="iA")
            iB_sb = epool.tile([128, slotsB // 16], i16, tag="iB")

            bounceA = dpool.tile([asplit, 128], bf16, tag="bA")
            bounceB = dpool.tile([bsplit, 128], bf16, tag="bB")
            tblA_sh = [dpool.tile([rows_a, 128], bf16, tag=f"tA{i}",
                                  name=f"tblA_sh{i}", addr_space="Shared")
                       for i in range(L - 1)]
            tblB_sh = [dpool.tile([rows_b, 128], bf16, tag=f"tB{i}",
                                  name=f"tblB_sh{i}", addr_space="Shared")
                       for i in range(L - 1)]
            stats_in = dpool.tile([2, d], fp32, tag="sin")
            stats_out = dpool.tile([2, d], fp32, tag="sout")

            nc.sync.dma_start(iA_sb[:], srcA[:])
            nc.sync.dma_start(iB_sb[:], srcB[:])
            for i in range(L):
                nc.sync.dma_start(W_sb[:, i * d:(i + 1) * d], Wbf_in[i, :, :])
            nc.sync.dma_start(bT_sb[:], bT_in[:])
            nc.sync.dma_start(gaT_sb[:], gammaT_in[:])
            nc.sync.dma_start(beT_sb[:], betaT_in[:])
            nc.sync.dma_start(prelu_sb[:], prelu_in[:])
            nc.sync.dma_start(degw_sb[:], degw_in[:])
            make_identity(nc, ident[:])
            nc.vector.memset(staging[:], 0.0)
            nc.gpsimd.load_library(library_config.mlp)

            gq = [0]  # global gather counter for queue rotation

            cflags = cfg["copy_flags"]

            for i in range(L):
                tbls = (tbl0A[:] if i == 0 else tblA_sh[i - 1][:],
                        tbl0B[:] if i == 0 else tblB_sh[i - 1][:])
                idxs = (iA_sb, iB_sb)
                Sins = (Sa_in, Sb_in)
                scheds = (schedA, schedB)
                tsubs = (cfg["subsA"], cfg["subsB"])

                if skip_smm:
                    nc.vector.memset(ragg[:], 0.0)
                for blk in cfg["msblocks"]:
                    nc.vector.memset(ragg[:, blk * blkn:(blk + 1) * blkn], 0.0)
                agg_ps = {}
                gs_c = gb_c = None

                def consume_stats(i):
                    gs_c = rpool.tile([d, 1], fp32, tag="gsc")
                    gb_c = rpool.tile([d, 1], fp32, tag="gbc")
                    mean_c = rpool.tile([d, 1], fp32, tag="meanc")
                    var_c = rpool.tile([d, 1], fp32, tag="varc")
                    tmp_c = rpool.tile([d, 1], fp32, tag="tmpc")
                    inv_n = 1.0 / float(n_nodes)
                    nc.sync.dma_start(stat2_sb[:],
                                      stats_out[:].rearrange("s d -> d s"))
                    nc.vector.tensor_scalar_mul(mean_c[:], stat2_sb[:, 0:1],
                                                inv_n)
                    nc.vector.tensor_scalar_mul(var_c[:], stat2_sb[:, 1:2],
                                                inv_n)
                    nc.vector.tensor_tensor(out=tmp_c[:], in0=mean_c[:],
                                            in1=mean_c[:], op=Alu.mult)
                    nc.vector.tensor_tensor(out=var_c[:], in0=var_c[:],
                                            in1=tmp_c[:], op=Alu.subtract)
                    nc.vector.tensor_scalar_add(var_c[:], var_c[:], BN_EPS)
                    nc.scalar.activation(tmp_c[:], var_c[:],
                                         mybir.ActivationFunctionType.Sqrt)
                    nc.vector.reciprocal(var_c[:], tmp_c[:])
                    nc.vector.tensor_tensor(out=gs_c[:],
                                            in0=gaT_sb[:, i - 1:i],
                                            in1=var_c[:], op=Alu.mult)
                    nc.vector.tensor_tensor(out=tmp_c[:], in0=mean_c[:],
                                            in1=gs_c[:], op=Alu.mult)
                    nc.vector.tensor_tensor(out=gb_c[:],
                                            in0=beT_sb[:, i - 1:i],
                                            in1=tmp_c[:], op=Alu.subtract)
                    return gs_c, gb_c

                for ev in cfg["events"]:
                    if ev[0] == "chunk":
                        s, c = ev[1], ev[2]
                        msg = mpool.tile([128, subc, 128], bf16, tag="msg")
                        nvalid = min(gchunk,
                                     max(0, tsubs[s] * 128 - c * gchunk))
                        if not skip_gather:
                            nc.gpsimd.dma_gather(
                                msg[:], tbls[s],
                                idxs[s][:, c * i16s:(c + 1) * i16s],
                                num_idxs=gchunk, num_idxs_reg=nvalid,
                                elem_size=128, queue_num=gq[0] % N_QUEUES,
                                single_packet=False)
                            gq[0] += 1
                        if skip_smm:
                            continue
                        S_sb = stpool.tile([128, subc, blkn], bf16, tag="S")
                        base = c * subc * blkn
                        nc.sync.dma_start(
                            S_sb[:],
                            Sins[s][:, base:base + subc * blkn]
                            .rearrange("e (s m) -> e s m", m=blkn))
                        for (j, blk, st, sp) in scheds[s][c]:
                            if st:
                                agg_ps[(s, blk)] = apool.tile(
                                    [d, blkn], fp32, name="aggps", tag="aggps")
                            nc.tensor.matmul(agg_ps[(s, blk)][:],
                                             lhsT=msg[:, j, 0:d],
                                             rhs=S_sb[:, j, :],
                                             start=st, stop=sp)
                            if sp:
                                lo = blk * blkn
                                if cflags[(s, blk)]:
                                    nc.vector.tensor_copy(
                                        ragg[:, lo:lo + blkn],
                                        agg_ps.pop((s, blk))[:])
                                else:
                                    nc.vector.tensor_tensor(
                                        out=ragg[:, lo:lo + blkn],
                                        in0=ragg[:, lo:lo + blkn],
                                        in1=agg_ps.pop((s, blk))[:],
                                        op=Alu.add)
                    elif ev[0] == "epi":
                        ch = ev[1]
                        if i > 0 and gs_c is None:
                            gs_c, gb_c = consume_stats(i)
                        lo = ch * 512
                        hi = min(npad, lo + 512)
                        cw = hi - lo
                        hbf = wpool.tile([d, 512], bf16, tag="hbf")
                        if i == 0:
                            nc.vector.tensor_copy(hbf[:, :cw], ragg[:, lo:hi])
                        else:
                            dgb = wpool.tile([d, 512], fp32, tag="dgb")
                            nc.vector.tensor_scalar_mul(
                                dgb[:, :cw], degw_sb[:, lo:hi], gb_c[:])
                            nc.vector.scalar_tensor_tensor(
                                out=hbf[:, :cw], in0=ragg[:, lo:hi],
                                scalar=gs_c[:], in1=dgb[:, :cw],
                                op0=Alu.mult, op1=Alu.add)
                        wps = wppool.tile([d, 512], fp32, tag="wps")
                        nc.tensor.matmul(wps[:, :cw],
                                         lhsT=W_sb[:, i * d:(i + 1) * d],
                                         rhs=hbf[:, :cw],
                                         start=True, stop=True)
                        t_ch = wpool.tile([d, 512], fp32, tag="tch")
                        u_ch = wpool.tile([d, 512], fp32, tag="uch")
                        nc.vector.tensor_scalar_add(t_ch[:, :cw], wps[:, :cw],
                                                    bT_sb[:, i:i + 1])
                        nc.vector.tensor_scalar_mul(u_ch[:, :cw],
                                                    t_ch[:, :cw],
                                                    prelu_sb[:d, i:i + 1])
                        if i > 0:
                            # residual h_{i-1} chunk from old p values
                            r_ch = wpool.tile([d, 512], fp32, tag="rch")
                            nc.vector.tensor_scalar(
                                out=r_ch[:, :cw], in0=p_sb[:, lo:hi],
                                scalar1=gs_c[:], scalar2=gb_c[:],
                                op0=Alu.mult, op1=Alu.add)
                            nc.vector.tensor_tensor(out=t_ch[:, :cw],
                                                    in0=t_ch[:, :cw],
                                                    in1=u_ch[:, :cw],
                                                    op=Alu.max)
                            nc.vector.scalar_tensor_tensor(
                                out=p_sb[:, lo:hi], in0=t_ch[:, :cw],
                                scalar=0.0, op0=Alu.add, in1=r_ch[:, :cw],
                                op1=Alu.add, accum_out=sumacc[:, ch:ch + 1])
                        else:
                            nc.vector.scalar_tensor_tensor(
                                out=p_sb[:, lo:hi], in0=t_ch[:, :cw],
                                scalar=0.0, op0=Alu.add, in1=u_ch[:, :cw],
                                op1=Alu.max, accum_out=sumacc[:, ch:ch + 1])
                        if ch == nec - 1 and npad > nsh:
                            # zero pad cols so stats sums see exact zeros
                            nc.vector.memset(p_sb[:, nsh:npad], 0.0)
                        sq = wpool.tile([d, 512], fp32, tag="sqch")
                        nc.vector.scalar_tensor_tensor(
                            out=sq[:, :cw], in0=p_sb[:, lo:hi], scalar=0.0,
                            op0=Alu.add, in1=p_sb[:, lo:hi], op1=Alu.mult,
                            accum_out=sqacc[:, ch:ch + 1])
                        # next-layer gather table: node-major transposes
                        if i < L - 1:
                            for t in range(lo // 128, hi // 128):
                                tr_ps = ppool.tile([128, d], fp32, tag="trps")
                                nc.tensor.transpose(
                                    tr_ps[:], p_sb[:, t * 128:(t + 1) * 128],
                                    ident[:])
                                nc.vector.tensor_copy(staging[:, t, 0:d],
                                                      tr_ps[:])
                    elif ev[0] == "agA" and i < L - 1:
                        ta = asplit // 128
                        nc.sync.dma_start(
                            bounceA[:].rearrange("(t p) m -> p t m", p=128),
                            staging[:, 0:ta, :])
                        nc.gpsimd.collective_compute(
                            "AllGather", mybir.AluOpType.bypass,
                            replica_groups=[list(range(n_cores))],
                            ins=[bounceA.opt()], outs=[tblA_sh[i].opt()])
                    elif ev[0] == "agB" and i < L - 1:
                        ta = asplit // 128
                        nc.sync.dma_start(
                            bounceB[:].rearrange("(t p) m -> p t m", p=128),
                            staging[:, ta:nt, :])
                        nc.gpsimd.collective_compute(
                            "AllGather", mybir.AluOpType.bypass,
                            replica_groups=[list(range(n_cores))],
                            ins=[bounceB.opt()], outs=[tblB_sh[i].opt()])

                # ---- stats reduce + AllReduce (consumed next layer) --------
                nc.vector.reduce_sum(stat_sb[:, 0:1], sumacc[:, 0:nec],
                                     axis=Ax.X)
                nc.vector.reduce_sum(stat_sb[:, 1:2], sqacc[:, 0:nec],
                                     axis=Ax.X)
                nc.sync.dma_start(stats_in[:].rearrange("s d -> d s"),
                                  stat_sb[:])
                nc.gpsimd.collective_compute(
                    "AllReduce", mybir.AluOpType.add,
                    replica_groups=[list(range(n_cores))],
                    ins=[stats_in.opt()], outs=[stats_out.opt()])

            # ---- final BN + output --------------------------------------
            gs_f = rpool.tile([d, 1], fp32, tag="gsf")
            gb_f = rpool.tile([d, 1], fp32, tag="gbf")
            mean_f = rpool.tile([d, 1], fp32, tag="meanf")
            var_f = rpool.tile([d, 1], fp32, tag="varf")
            tmp_f = rpool.tile([d, 1], fp32, tag="tmpf")
            inv_n = 1.0 / float(n_nodes)
            nc.sync.dma_start(stat2_sb[:],
                              stats_out[:].rearrange("s d -> d s"))
            nc.vector.tensor_scalar_mul(mean_f[:], stat2_sb[:, 0:1], inv_n)
            nc.vector.tensor_scalar_mul(var_f[:], stat2_sb[:, 1:2], inv_n)
            nc.vector.tensor_tensor(out=tmp_f[:], in0=mean_f[:],
                                    in1=mean_f[:], op=Alu.mult)
            nc.vector.tensor_tensor(out=var_f[:], in0=var_f[:],
                                    in1=tmp_f[:], op=Alu.subtract)
            nc.vector.tensor_scalar_add(var_f[:], var_f[:], BN_EPS)
            nc.scalar.activation(tmp_f[:], var_f[:],
                                 mybir.ActivationFunctionType.Sqrt)
            nc.vector.reciprocal(var_f[:], tmp_f[:])
            nc.vector.tensor_tensor(out=gs_f[:], in0=gaT_sb[:, L - 1:L],
                                    in1=var_f[:], op=Alu.mult)
            nc.vector.tensor_tensor(out=tmp_f[:], in0=mean_f[:],
                                    in1=gs_f[:], op=Alu.mult)
            nc.vector.tensor_tensor(out=gb_f[:], in0=beT_sb[:, L - 1:L],
                                    in1=tmp_f[:], op=Alu.subtract)
            nc.vector.tensor_scalar(out=ragg[:], in0=p_sb[:],
                                    scalar1=gs_f[:], scalar2=gb_f[:],
                                    op0=Alu.mult, op1=Alu.add)

            outv = staging[:].bitcast(fp32)   # [128, nt, 64] fp32 view
            for t in range(nt):
                tr_ps = ppool.tile([128, d], fp32, tag="trps")
                nc.tensor.transpose(tr_ps[:], ragg[:, t * 128:(t + 1) * 128],
                                    ident[:])
                nc.vector.tensor_copy(outv[:, t, :], tr_ps[:])
            nc.sync.dma_start(out_ext[:].rearrange("(t p) d -> p t d", p=128),
                              outv)
    nc.compile()
    return nc


# ----------------------------------------------------------------------------
# Entry point
# ----------------------------------------------------------------------------

def kernel(x, edge_src, edge_dst, edge_weight, W, b, prelu_a,
           bn_gamma, bn_beta):
    from concourse.bass_utils import run_bass_kernel_spmd

    x = np.asarray(x)
    n = x.shape[0]
    nsh = n // N_CORES
    in_maps, cfg = _preprocess(x, edge_src, edge_dst, edge_weight, W, b,
                               prelu_a, bn_gamma, bn_beta,
                               N_CORES, nsh, GCHUNK, BLKN)
    nc = _build_nc(cfg)
    trace = bool(int(os.environ.get("GCN_TRACE", "0")))
    if trace:
        try:
            import antenv.axon_hooks  # noqa: F401
        except ImportError:
            trace = False
    res = run_bass_kernel_spmd(nc, in_maps, core_ids=list(range(N_CORES)),
                               trace=trace)
    LAST_RUN["results"] = res
    LAST_RUN["cfg"] = cfg
    LAST_RUN["nc"] = nc
    LAST_RUN["in_maps"] = in_maps
    slot = cfg["slot_of_node"]
    all_res = np.stack([np.asarray(res.results[r]["out"])
                        for r in range(N_CORES)])      # [cores, npad, d]
    out = all_res[slot // cfg["npad"], slot % cfg["npad"]]
    return out.astype(np.float32)


def measure_exec_ns(nc, in_maps, n_reps=10):
    """Steady-state device-time estimate: pre-staged device inputs; marginal
    (slope) wall time of k back-to-back NEFF executions, amortizing the
    axon tunnel dispatch overhead."""
    import time
    import jax
    import concourse.mybir as mybir
    from jax.sharding import Mesh, PartitionSpec, NamedSharding
    from jax.experimental.shard_map import shard_map
    from concourse import bass2jax

    n_cores = len(in_maps)
    partition_name = (nc.partition_id_tensor.name
                      if nc.partition_id_tensor else None)
    in_names, out_names, out_avals = [], [], []
    for alloc in nc.m.functions[0].allocations:
        if not isinstance(alloc, mybir.MemoryLocationSet):
            continue
        name = alloc.memorylocations[0].name
        if alloc.kind == "ExternalInput":
            if name != partition_name:
                in_names.append(name)
        elif alloc.kind == "ExternalOutput":
            out_names.append(name)
            out_avals.append(jax.core.ShapedArray(
                tuple(alloc.tensor_shape), mybir.dt.np(alloc.dtype)))
    n_params = len(in_names)
    all_in = list(in_names) + list(out_names)
    if partition_name is not None:
        all_in.append(partition_name)

    def _body(*args):
        operands = list(args)
        if partition_name is not None:
            operands.append(bass2jax.partition_id_tensor())
        outs = bass2jax._bass_exec_p.bind(
            *operands, out_avals=tuple(out_avals), in_names=tuple(all_in),
            out_names=tuple(out_names), lowering_input_output_aliases=(),
            sim_require_finite=True, sim_require_nnan=True, nc=nc)
        return tuple(outs)

    devices = jax.devices()[:n_cores]
    mesh = Mesh(np.asarray(devices), ("core",))
    nin = n_params + len(out_names)
    fn = jax.jit(shard_map(_body, mesh=mesh,
                           in_specs=(PartitionSpec("core"),) * nin,
                           out_specs=(PartitionSpec("core"),) * len(out_names),
                           check_rep=False))
    sh = NamedSharding(mesh, PartitionSpec("core"))
    dev_in = [jax.device_put(
        np.concatenate([np.asarray(in_maps[c][k]) for c in range(n_cores)],
                       axis=0), sh) for k in in_names]
    dev_zero = [jax.device_put(
        np.zeros((n_cores * a.shape[0], *a.shape[1:]), a.dtype), sh)
        for a in out_avals]
    out = fn(*dev_in, *dev_zero)
    jax.block_until_ready(out)

    def best_of(k, reps):
        best = 1e9
        for _ in range(reps):
            t0 = time.perf_counter()
            last = None
            for _ in range(k):
                last = fn(*dev_in, *dev_zero)
            jax.block_until_ready(last)
            best = min(best, time.perf_counter() - t0)
        return best

    # slopes between amortized queue depths; contention only adds time,
    # so the minimum pairwise marginal is the least-contaminated estimate
    t8 = best_of(8, 4)
    t16 = best_of(16, 4)
    t32 = best_of(32, 4)
    cands = [(t32 - t8) / 24, (t32 - t16) / 16, (t16 - t8) / 8]
    pos = [c for c in cands if c > 0]
    marginal = min(pos) if pos else abs((t32 - t8) / 24)
    times = [t8, t16, t32]
    return int(marginal * 1e9), times
